# revision 21
# baseline (speedup 1.0000x reference)
"""CrossAttentionTransformerBlock on 8 TRN2 NeuronCores (Bass/Tile), fp16.

Sharding: core 2b+s handles (batch b, stream s); s=0 -> x, s=1 -> src.
Self-attn + MLP are stream-local; the bidirectional cross-attention exchanges
(keys, values) between the pair (2b, 2b+1) via two chunked pairwise
AllReduces (partner = sum - mine), pipelined behind the tail of the
self-attention block so the wire time is hidden.

All activations/weights are fp16 (PSUM accumulation stays fp32) so every
matmul streams at 1 cycle/row and weight loads use the fast FWL path; the
previous fp32r version ran in fp32_mode=HIGH at less than half throughput
and kept the PE clock throttled. LN scale/bias are folded into the adjacent
projection weights on the host; softmax has no max-subtraction (logits are
small by construction); denominators come from a ones-augmented column of V
in the AV matmul and are applied via a broadcast matmul + wide multiply
(no single-partition reciprocals).
"""

import numpy as np

import concourse.bacc as bacc
import concourse.bass as bass
import concourse.mybir as mybir
import concourse.tile as tile
from concourse.bass_utils import run_bass_kernel_spmd

F32 = mybir.dt.float32
F16 = mybir.dt.float16
AF = mybir.ActivationFunctionType
OP = mybir.AluOpType

B, T, C, H, HD = 4, 1024, 768, 12, 64
HID = 4 * C
EPS = 1e-6
SCALE = HD ** -0.5
NT = T // 128       # 8 token tiles
KC = C // 128       # 6 feature chunks
NH = T // 512       # 2 token halves
NHT = HID // 128    # 24 hidden tiles
N_CORES = 8
GROUPS = [[0, 1], [2, 3], [4, 5], [6, 7]]

# exchange chunk: K-half [C, 512] + V-half [4 x 128, 780], fp16
KCH = C * 512                 # elems of K per chunk
VCH = 4 * 128 * 780           # elems of V per chunk
CCN = KCH + VCH

_CACHE = {}


def _emit(nc):
    dp = nc.declare_dram_parameter
    tok_d = dp("tok", [T, C], F32, isOutput=False)
    ident_d = dp("ident", [128, 128], F16, isOutput=False)
    ones_d = dp("ones", [128, 128], F16, isOutput=False)
    wqk_d = dp("wqk", [C, 2 * C], F16, isOutput=False)
    wvs_d = dp("wvs", [C, C], F16, isOutput=False)
    wproj_d = dp("wproj", [C, C], F16, isOutput=False)
    wab_d = dp("wab", [C, 2 * C], F16, isOutput=False)
    wvc_d = dp("wvc", [C, C], F16, isOutput=False)
    wcp_d = dp("wcp", [C, C], F16, isOutput=False)
    wm1_d = dp("wm1", [C, HID], F16, isOutput=False)
    wm2_d = dp("wm2", [HID, C], F16, isOutput=False)
    bqk_d = dp("bqk", [128, 12], F32, isOutput=False)
    bab_d = dp("bab", [128, 12], F32, isOutput=False)
    bproj_d = dp("bproj", [128, 6], F32, isOutput=False)
    bcp_d = dp("bcp", [128, 6], F32, isOutput=False)
    bm1_d = dp("bm1", [128, 24], F32, isOutput=False)
    bm2_d = dp("bm2", [128, 6], F32, isOutput=False)
    out_d = dp("out_tok", [T, C], F32, isOutput=True)

    with tile.TileContext(nc) as tc:
        import contextlib
        es = contextlib.ExitStack()
        with es:
            es.enter_context(nc.allow_low_precision(
                reason="fp16 kernel by design; matmul/LN accumulate in f32"))
            pP = es.enter_context(tc.tile_pool(name="pP", bufs=1))
            pROW = es.enter_context(tc.tile_pool(name="pROW", bufs=4))
            pSQ = es.enter_context(tc.tile_pool(name="pSQ", bufs=2))
            pVAR = es.enter_context(tc.tile_pool(name="pVAR", bufs=1))
            pPT = es.enter_context(tc.tile_pool(name="pPT", bufs=3))
            pRC = es.enter_context(tc.tile_pool(name="pRC", bufs=1))
            psA = es.enter_context(
                tc.tile_pool(name="psA", bufs=3, space="PSUM"))
            dram = es.enter_context(
                tc.tile_pool(name="dram", bufs=1, space="DRAM"))

            ident = pP.tile([128, 128], F16, tag="ident")
            onesm = pP.tile([128, 128], F16, tag="onesm")
            nc.sync.dma_start(out=ident[:], in_=ident_d[:])
            nc.sync.dma_start(out=onesm[:], in_=ones_d[:])
            eps_t = pP.tile([128, 1], F32, tag="epst")
            nc.vector.memset(eps_t[:], EPS)
            den_tall = [pP.tile([96, T], F16, tag=f"dent{i}",
                                name=f"dent{i}") for i in range(4)]
            for i in range(4):
                nc.vector.memset(den_tall[i][:], 1.0)

            # bias packs
            def bias_pack(b_dram, n):
                bt = pP.tile([128, n], F32, tag=f"B{b_dram.name}", name="bp")
                nc.sync.dma_start(out=bt[:], in_=b_dram[:, 0:n])
                return bt

            bqk_t = bias_pack(bqk_d, 12)
            bab_t = bias_pack(bab_d, 12)
            bproj_t = bias_pack(bproj_d, 6)
            bcp_t = bias_pack(bcp_d, 6)
            bm1_t = bias_pack(bm1_d, 24)
            bm2_t = bias_pack(bm2_d, 6)

            # residual stream + LN output, persistent
            xT = [pP.tile([128, T], F16, tag=f"XT{i}", name=f"xT{i}") for i in range(KC)]
            xh = [pP.tile([128, T], F16, tag=f"XH{i}", name=f"xh{i}") for i in range(KC)]

            # ---- weight slab loaders ----
            def load_slabs(pool, w_dram, width, tag):
                out = []
                for kc in range(KC):
                    s = pool.tile([128, width], F16, tag=f"{tag}{kc}", name=f"w{tag}{kc}")
                    nc.sync.dma_start(
                        out=s[:], in_=w_dram[kc * 128:(kc + 1) * 128, :])
                    out.append(s)
                return out

            # small psum pool: LN stats/broadcasts + 128x128 transposes
            psB = es.enter_context(
                tc.tile_pool(name="psB", bufs=2, space="PSUM"))

            # prefetch-pool stack (LIFO closes; DMAs are emitted later)
            pWpj_cm = tc.tile_pool(name="pWpj", bufs=1)
            pWpj = pWpj_cm.__enter__()
            pWc_cm = tc.tile_pool(name="pWc", bufs=1)
            pWc = pWc_cm.__enter__()
            pOT_cm = tc.tile_pool(name="pOT", bufs=1)
            pOT = pOT_cm.__enter__()
            pWqv_cm = tc.tile_pool(name="pWqv", bufs=1)
            pWqv = pWqv_cm.__enter__()
            pQK_cm = tc.tile_pool(name="pQK", bufs=1)
            pQK = pQK_cm.__enter__()
            pVA_cm = tc.tile_pool(name="pVA", bufs=1)
            pVA = pVA_cm.__enter__()

            # ---- phase 0: tokens -> feature-major fp16 ----
            pTOK_cm = tc.tile_pool(name="pTOK", bufs=2)
            pTOK = pTOK_cm.__enter__()
            pTOKB_cm = tc.tile_pool(name="pTOKB", bufs=2)
            pTOKB = pTOKB_cm.__enter__()
            for tt in range(NT):
                t_ = pTOK.tile([128, C], F32, tag="TOK", name="tok_t")
                nc.sync.dma_start(out=t_[:],
                                  in_=tok_d[tt * 128:(tt + 1) * 128, :])
                tb = pTOKB.tile([128, C], F16, tag="TOKB", name="tokb")
                nc.vector.tensor_copy(tb[:], t_[:])
                for cc in range(KC):
                    pt = psB.tile([128, 128], F16, tag="LN", name="pt")
                    nc.tensor.transpose(
                        pt[:], tb[:, cc * 128:(cc + 1) * 128], ident[:])
                    nc.vector.tensor_copy(
                        xT[cc][:, tt * 128:(tt + 1) * 128], pt[:])
            # weight prefetch: self first, then cross (queue order = need order)
            wqks = load_slabs(pWqv, wqk_d, 2 * C, "WQK")
            wvss = load_slabs(pWqv, wvs_d, C, "WVS")
            wprojs = load_slabs(pWpj, wproj_d, C, "WPJ")
            wabs = load_slabs(pWc, wab_d, 2 * C, "WAB")
            wvcs = load_slabs(pWc, wvc_d, C, "WVC")
            wcps = load_slabs(pWc, wcp_d, C, "WCP")
            pTOKB_cm.__exit__(None, None, None)
            pTOK_cm.__exit__(None, None, None)

            # ---- LN of one token half: writes xh[:][:, hsl] ----
            def ln_half(hf):
                hsl = slice(hf * 512, (hf + 1) * 512)
                ps_mu = psB.tile([1, 512], F32, tag="LN", name="psmu")
                ps_sq = psB.tile([1, 512], F32, tag="LN", name="pssq")
                for kc in range(KC):
                    sqt = pSQ.tile([128, 512], F16, tag="SQT", name="sqt")
                    nc.vector.tensor_tensor(
                        sqt[:], xT[kc][:, hsl], xT[kc][:, hsl], OP.mult)
                    nc.tensor.matmul(
                        ps_mu[0:1, :], onesm[:, 0:1], xT[kc][:, hsl],
                        start=(kc == 0), stop=(kc == KC - 1))
                    nc.tensor.matmul(
                        ps_sq[0:1, :], onesm[:, 0:1], sqt[:],
                        start=(kc == 0), stop=(kc == KC - 1))
                mu_r = pROW.tile([1, 512], F16, tag="ROW", name="mu_r")
                sq_r = pROW.tile([1, 512], F16, tag="ROW", name="sq_r")
                nc.scalar.activation(mu_r[:], ps_mu[0:1, :], AF.Identity,
                                     scale=1.0 / C)
                nc.scalar.activation(sq_r[:], ps_sq[0:1, :], AF.Identity,
                                     scale=1.0 / C)
                mean_b = psB.tile([128, 512], F32, tag="LN", name="meanb")
                m2_b = psB.tile([128, 512], F32, tag="LN", name="m2b")
                nc.tensor.matmul(mean_b[:], onesm[0:1, :], mu_r[0:1, :],
                                 start=True, stop=True)
                nc.tensor.matmul(m2_b[:], onesm[0:1, :], sq_r[0:1, :],
                                 start=True, stop=True)
                meansb = pVAR.tile([128, 512], F32, tag="MEAN", name="meansb")
                nc.vector.tensor_copy(meansb[:], mean_b[:])
                vs = pVAR.tile([128, 512], F32, tag="VAR", name="vs")
                nc.vector.tensor_tensor(vs[:], meansb[:], meansb[:], OP.mult)
                nc.vector.tensor_tensor(vs[:], m2_b[:], vs[:], OP.subtract)
                rstd = pVAR.tile([128, 512], F16, tag="RSTD", name="rstd")
                nc.scalar.activation(vs[:], vs[:], AF.Ln, bias=eps_t[:])
                nc.scalar.activation(rstd[:], vs[:], AF.Exp, scale=-0.5)
                for kc in range(KC):
                    nc.vector.tensor_tensor(
                        xh[kc][:, hsl], xT[kc][:, hsl], meansb[:],
                        OP.subtract)
                    nc.vector.tensor_tensor(
                        xh[kc][:, hsl], xh[kc][:, hsl], rstd[:], OP.mult)

            # ---- feature-major projection of xh into dst tiles ----
            def proj_fm(wslabs, col0, nout, bias_t, bidx0, dst):
                for ot in range(nout):
                    pp = psA.tile([128, T], F32, tag="PS", name="pp")
                    for hf in range(NH):
                        hsl = slice(hf * 512, (hf + 1) * 512)
                        for kc in range(KC):
                            nc.tensor.matmul(
                                pp[:, hsl],
                                wslabs[kc][:, col0 + ot * 128:
                                           col0 + (ot + 1) * 128],
                                xh[kc][:, hsl],
                                start=(kc == 0), stop=(kc == KC - 1))
                    nc.vector.tensor_scalar(
                        dst[ot][:], pp[:], bias_t[:, bidx0 + ot:bidx0 + ot + 1],
                        None, op0=OP.add)

            # ---- token-major ones-augmented V for token tile mt ----
            def proj_v(wslabs, mt, dst):
                pp = psA.tile([128, T], F32, tag="PS", name="ppv")
                for ck in range(2):
                    sl = slice(ck * 512, min((ck + 1) * 512, C))
                    for kc in range(KC):
                        nc.tensor.matmul(
                            pp[:, sl],
                            xh[kc][:, mt * 128:(mt + 1) * 128],
                            wslabs[kc][:, sl],
                            start=(kc == 0), stop=(kc == KC - 1))
                va3 = dst[:].rearrange("p (h e) -> p h e", e=65)
                nc.vector.tensor_copy(
                    va3[:, :, 0:64],
                    pp[:, 0:C].rearrange("p (h e) -> p h e", e=64))
                nc.vector.memset(va3[:, :, 64:65], 1.0)

            # ---- attention: q/k feature-major accessors + V tiles ----
            def norm_head(den_ap, ct, ro, oT):
                dp_ = den_ap.base_partition()
                psR = psA.tile([128, T], F32, tag="PS", name="psR")
                for hf in range(NH):
                    hsl = slice(hf * 512, (hf + 1) * 512)
                    nc.tensor.matmul(psR[0:64, hsl],
                                     onesm[dp_:dp_ + 1, 0:64],
                                     den_ap[:, hsl], start=True, stop=True)
                rcpw = pRC.tile([128, T], F32, tag="RCW", name="rcpw")
                nc.vector.reciprocal(rcpw[ro:ro + 64, :], psR[0:64, :])
                nc.vector.tensor_tensor(
                    oT[ct][ro:ro + 64, :], oT[ct][ro:ro + 64, :],
                    rcpw[ro:ro + 64, :], OP.mult)

            def attention(qt_of, kt_of, va_of, pOT):
                oT = [pOT.tile([128, T], F16, tag=f"OT{i}", name=f"oT{i}")
                      for i in range(KC)]
                for h in range(H):
                    ct, ro = divmod(h * HD, 128)
                    psO = psA.tile([128, T], F32, tag="PS", name="psO")
                    for mt in range(NT):
                        psS = psA.tile([128, T], F32, tag="PS", name="psS")
                        for hf in range(NH):
                            hsl = slice(hf * 512, (hf + 1) * 512)
                            nc.tensor.matmul(
                                psS[:, hsl], kt_of(ct, ro, mt),
                                qt_of(ct, ro, hsl),
                                start=True, stop=True)
                        pT = pPT.tile([128, T], F16, tag="PT", name="pT")
                        nc.scalar.activation(pT[:], psS[:], AF.Exp)
                        for hf in range(NH):
                            hsl = slice(hf * 512, (hf + 1) * 512)
                            nc.tensor.matmul(
                                psO[0:65, hsl],
                                va_of(mt)[:, h * 65:(h + 1) * 65],
                                pT[:, hsl],
                                start=(mt == 0), stop=(mt == NT - 1))
                    den = pROW.tile([1, T], F16, tag="DEN", name="den")
                    nc.scalar.activation(den[:], psO[64:65, :], AF.Identity)
                    nc.vector.tensor_copy(oT[ct][ro:ro + 64, :], psO[0:64, :])
                    norm_head(den[0:1, :], ct, ro, oT)
                return oT

            # ---- projection of oT + residual add into xT (half hf) ----
            def proj_res_half(wslabs, bias_t, oT, hf):
                hsl = slice(hf * 512, (hf + 1) * 512)
                for ot in range(KC):
                    pp = psA.tile([128, T], F32, tag="PS", name="ppr")
                    for kc in range(KC):
                        nc.tensor.matmul(
                            pp[:, 0:512],
                            wslabs[kc][:, ot * 128:(ot + 1) * 128],
                            oT[kc][:, hsl],
                            start=(kc == 0), stop=(kc == KC - 1))
                    nc.vector.scalar_tensor_tensor(
                        out=xT[ot][:, hsl], in0=pp[:, 0:512],
                        scalar=bias_t[:, ot:ot + 1], in1=xT[ot][:, hsl],
                        op0=OP.add, op1=OP.add)

            # ================= self-attention =================
            ln_half(0)
            ln_half(1)
            qT = [pQK.tile([128, T], F16, tag=f"QT{i}", name=f"qT{i}") for i in range(KC)]
            kT = [pQK.tile([128, T], F16, tag=f"KT{i}", name=f"kT{i}") for i in range(KC)]
            proj_fm(wqks, 0, KC, bqk_t, 0, qT)
            proj_fm(wqks, C, KC, bqk_t, 6, kT)
            va_s = [pVA.tile([128, 780], F16, tag=f"VA{i}", name=f"va{i}") for i in range(NT)]
            for mt in range(NT):
                proj_v(wvss, mt, va_s[mt])

            oT = attention(
                lambda ct, ro, hsl: qT[ct][ro:ro + HD, hsl],
                lambda ct, ro, mt: kT[ct][ro:ro + HD, mt * 128:(mt + 1) * 128],
                lambda mt: va_s[mt],
                pOT)

            pVA_cm.__exit__(None, None, None)
            pQK_cm.__exit__(None, None, None)
            pWqv_cm.__exit__(None, None, None)

            # ============ cross-attention prep, chunked exchange ============
            pBT_cm = tc.tile_pool(name="pBT", bufs=1)
            pBT = pBT_cm.__enter__()
            pVC_cm = tc.tile_pool(name="pVC", bufs=1)
            pVC = pVC_cm.__enter__()

            bTc = [[pBT.tile([128, 512], F16, tag=f"BT{c}_{i}", name=f"bT{c}_{i}")
                    for i in range(KC)] for c in range(2)]
            bPc = [[pBT.tile([128, 512], F16, tag=f"BP{c}_{i}", name=f"bP{c}_{i}")
                    for i in range(KC)] for c in range(2)]
            aT = [pBT.tile([128, T], F16, tag=f"AT{i}", name=f"aT{i}") for i in range(KC)]
            va_c = [pVC.tile([128, 780], F16, tag=f"VC{i}", name=f"vc{i}") for i in range(NT)]
            vP = [pVC.tile([128, 780], F16, tag=f"VP{i}", name=f"vp{i}") for i in range(NT)]

            cc_in = [dram.tile([CCN], F16, tag=f"cci{c}", name=f"cci{c}") for c in range(2)]
            cc_out = [dram.tile([CCN], F16, tag=f"cco{c}", name=f"cco{c}")
                      for c in range(2)]

            for hf in range(NH):
                hsl = slice(hf * 512, (hf + 1) * 512)
                proj_res_half(wprojs, bproj_t, oT, hf)
                ln_half(hf)
                # bT half: keys this core provides to its partner
                for ot in range(KC):
                    pp = psA.tile([128, T], F32, tag="PS", name="ppb")
                    for kc in range(KC):
                        nc.tensor.matmul(
                            pp[:, 0:512],
                            wabs[kc][:, C + ot * 128:C + (ot + 1) * 128],
                            xh[kc][:, hsl],
                            start=(kc == 0), stop=(kc == KC - 1))
                    nc.vector.tensor_scalar(
                        bTc[hf][ot][:], pp[:, 0:512],
                        bab_t[:, 6 + ot:7 + ot], None, op0=OP.add)
                    nc.sync.dma_start(
                        out=cc_in[hf][ot * 65536:(ot + 1) * 65536]
                        .rearrange("(p c) -> p c", c=512),
                        in_=bTc[hf][ot][:])
                # V half (token tiles of this half)
                for mt in range(hf * 4, hf * 4 + 4):
                    proj_v(wvcs, mt, va_c[mt])
                    nc.sync.dma_start(
                        out=cc_in[hf][KCH + (mt - hf * 4) * 99840:
                                      KCH + (mt - hf * 4 + 1) * 99840]
                        .rearrange("(p c) -> p c", c=780),
                        in_=va_c[mt][:])
                nc.gpsimd.collective_compute(
                    "AllReduce", OP.add, replica_groups=GROUPS,
                    ins=[cc_in[hf].opt()], outs=[cc_out[hf].opt()])

            # queries (overlap with the exchange)
            proj_fm(wabs, 0, KC, bab_t, 0, aT)

            # partner K/V: subtract own contribution from the pair sum
            pRS_cm = tc.tile_pool(name="pRS", bufs=4)
            pRS = pRS_cm.__enter__()
            for c in range(2):
                for ot in range(KC):
                    st = pRS.tile([128, 512], F16, tag="RS", name="st")
                    nc.sync.dma_start(
                        out=st[:],
                        in_=cc_out[c][ot * 65536:(ot + 1) * 65536]
                        .rearrange("(p c) -> p c", c=512))
                    nc.gpsimd.tensor_tensor(
                        bPc[c][ot][:], st[:], bTc[c][ot][:], OP.subtract)
                for mt in range(c * 4, c * 4 + 4):
                    sv = pRS.tile([128, 780], F16, tag="RSV", name="sv")
                    nc.sync.dma_start(
                        out=sv[:],
                        in_=cc_out[c][KCH + (mt - c * 4) * 99840:
                                      KCH + (mt - c * 4 + 1) * 99840]
                        .rearrange("(p c) -> p c", c=780))
                    nc.gpsimd.tensor_tensor(
                        vP[mt][:], sv[:], va_c[mt][:], OP.subtract)

            pOTc_cm = tc.tile_pool(name="pOTc", bufs=1)
            pOTc = pOTc_cm.__enter__()
            oTc = [pOTc.tile([128, T], F16, tag=f"OC{i}", name=f"oC{i}")
                   for i in range(KC)]

            def cross_chunk(c):
                for h in range(H):
                    ct, ro = divmod(h * HD, 128)
                    psO = psA.tile([128, T], F32, tag="PS", name="psOc")
                    for mt in range(c * 4, c * 4 + 4):
                        psS = psA.tile([128, T], F32, tag="PS", name="psSc")
                        for hf in range(NH):
                            hsl = slice(hf * 512, (hf + 1) * 512)
                            nc.tensor.matmul(
                                psS[:, hsl],
                                bPc[c][ct][ro:ro + HD,
                                           (mt % 4) * 128:(mt % 4 + 1) * 128],
                                aT[ct][ro:ro + HD, hsl],
                                start=True, stop=True)
                        pT = pPT.tile([128, T], F16, tag="PT", name="pTc")
                        nc.scalar.activation(pT[:], psS[:], AF.Exp)
                        for hf in range(NH):
                            hsl = slice(hf * 512, (hf + 1) * 512)
                            nc.tensor.matmul(
                                psO[0:65, hsl],
                                vP[mt][:, h * 65:(h + 1) * 65],
                                pT[:, hsl],
                                start=(mt % 4 == 0), stop=(mt % 4 == 3))
                    dt_, dr = den_tall[h // 3], 32 * (h % 3)
                    if c == 0:
                        nc.vector.tensor_copy(oTc[ct][ro:ro + 64, :],
                                              psO[0:64, :])
                        nc.vector.tensor_copy(dt_[dr:dr + 1, :],
                                               psO[64:65, :])
                    else:
                        nc.vector.tensor_tensor(
                            oTc[ct][ro:ro + 64, :], oTc[ct][ro:ro + 64, :],
                            psO[0:64, :], OP.add)
                        nc.vector.tensor_tensor(
                            dt_[dr:dr + 1, :], dt_[dr:dr + 1, :],
                            psO[64:65, :], OP.add)
                        norm_head(dt_[dr:dr + 1, :], ct, ro, oTc)

            cross_chunk(0)
            cross_chunk(1)

            for hf in range(NH):
                proj_res_half(wcps, bcp_t, oTc, hf)

            # free self/cross phase SBUF before MLP (reverse open order)
            pOTc_cm.__exit__(None, None, None)
            pRS_cm.__exit__(None, None, None)
            pVC_cm.__exit__(None, None, None)
            pBT_cm.__exit__(None, None, None)
            pOT_cm.__exit__(None, None, None)
            pWc_cm.__exit__(None, None, None)
            pWpj_cm.__exit__(None, None, None)

            # ================= MLP =================
            pWm1_cm = tc.tile_pool(name="pWm1", bufs=1)
            pWm1 = pWm1_cm.__enter__()
            wm1s = load_slabs(pWm1, wm1_d, HID, "WM1")
            pWm2_cm = tc.tile_pool(name="pWm2", bufs=1)
            pWm2 = pWm2_cm.__enter__()
            wm2s = []
            for ht in range(NHT):
                s = pWm2.tile([128, C], F16, tag=f"WM2_{ht}", name=f"wm2_{ht}")
                nc.sync.dma_start(
                    out=s[:], in_=wm2_d[ht * 128:(ht + 1) * 128, :])
                wm2s.append(s)

            pHT_cm = tc.tile_pool(name="pHT", bufs=1)
            pHT = pHT_cm.__enter__()
            hT = [pHT.tile([128, T], F16, tag=f"HT{i}", name=f"hT{i}") for i in range(NHT)]

            ln_half(0)
            ln_half(1)
            for ht in range(NHT):
                pp = psA.tile([128, T], F32, tag="PS", name="pph")
                for hf in range(NH):
                    hsl = slice(hf * 512, (hf + 1) * 512)
                    for kc in range(KC):
                        nc.tensor.matmul(
                            pp[:, hsl],
                            wm1s[kc][:, ht * 128:(ht + 1) * 128],
                            xh[kc][:, hsl],
                            start=(kc == 0), stop=(kc == KC - 1))
                nc.scalar.activation(hT[ht][:], pp[:], AF.Gelu,
                                     bias=bm1_t[:, ht:ht + 1], scale=1.0)
            for ot in range(KC):
                pp = psA.tile([128, T], F32, tag="PS", name="pp2")
                for hf in range(NH):
                    hsl = slice(hf * 512, (hf + 1) * 512)
                    for ht in range(NHT):
                        nc.tensor.matmul(
                            pp[:, hsl],
                            wm2s[ht][:, ot * 128:(ot + 1) * 128],
                            hT[ht][:, hsl],
                            start=(ht == 0), stop=(ht == NHT - 1))
                nc.vector.scalar_tensor_tensor(
                    out=xT[ot][:], in0=pp[:], scalar=bm2_t[:, ot:ot + 1],
                    in1=xT[ot][:], op0=OP.add, op1=OP.add)

            # ---- output transpose + DMA ----
            pOUT_cm = tc.tile_pool(name="pOUT", bufs=2)
            pOUT = pOUT_cm.__enter__()
            for tt in range(NT):
                ot_sb = pOUT.tile([128, C], F32, tag="OUTT", name="ot_sb")
                for cc in range(KC):
                    pt = psB.tile([128, 128], F16, tag="LN", name="pt2")
                    nc.tensor.transpose(
                        pt[:], xT[cc][:, tt * 128:(tt + 1) * 128], ident[:])
                    nc.vector.tensor_copy(
                        ot_sb[:, cc * 128:(cc + 1) * 128], pt[:])
                nc.sync.dma_start(out=out_d[tt * 128:(tt + 1) * 128, :],
                                  in_=ot_sb[:])
            pOUT_cm.__exit__(None, None, None)
            pHT_cm.__exit__(None, None, None)
            pWm2_cm.__exit__(None, None, None)
            pWm1_cm.__exit__(None, None, None)

    nc.compile()
    return nc


def _build():
    if "nc" not in _CACHE:
        nc = bacc.Bacc("TRN2", target_bir_lowering=False)
        _CACHE["nc"] = _emit(nc)
    return _CACHE["nc"]


def _fold_ln(w, ln_w, ln_b):
    """w [out, in]; returns (w', b') with LN scale/bias folded in."""
    w = np.asarray(w, np.float64)
    wf = w * np.asarray(ln_w, np.float64)[None, :]
    bf = w @ np.asarray(ln_b, np.float64)
    return wf, bf


def _pack_bias(b, n):
    return np.ascontiguousarray(
        np.asarray(b, np.float64).reshape(n, 128).T, np.float32)


def _core_inputs(s, tok, p):
    sfx = "" if s == 0 else "s"
    wqkv, bqkv = _fold_ln(p["w_qkv" + ("" if s == 0 else "_s")],
                          p[f"ln1{sfx}_w"], p[f"ln1{sfx}_b"])
    wqkv = wqkv.copy()
    wqkv[:C] *= SCALE
    bqkv = bqkv.copy()
    bqkv[:C] *= SCALE
    wproj = np.asarray(p["w_proj" + ("" if s == 0 else "_s")], np.float64)
    bproj = np.asarray(p["b_proj" + ("" if s == 0 else "_s")], np.float64) \
        + wproj @ bqkv[2 * C:]
    lncw = p["lnc_w" if s == 0 else "lncs_w"]
    lncb = p["lnc_b" if s == 0 else "lncs_b"]
    wqk, bqk_ = _fold_ln(p["w_qk" if s == 0 else "w_qk_src"], lncw, lncb)
    wqk3 = wqk.reshape(H, 2 * HD, C)
    bqk3 = bqk_.reshape(H, 2 * HD)
    if s == 0:
        A, Ab = wqk3[:, :HD] * SCALE, bqk3[:, :HD] * SCALE
        Bm, Bb = wqk3[:, HD:] * SCALE, bqk3[:, HD:] * SCALE
    else:
        A, Ab = wqk3[:, HD:], bqk3[:, HD:]
        Bm, Bb = wqk3[:, :HD], bqk3[:, :HD]
    wab = np.concatenate([A.reshape(C, C), Bm.reshape(C, C)], axis=0)
    bab = np.concatenate([Ab.reshape(C), Bb.reshape(C)], axis=0)
    wvc, bvc = _fold_ln(p["w_v" if s == 0 else "w_v_src"], lncw, lncb)
    wcp = np.asarray(p["w_cp" if s == 0 else "w_cp_src"], np.float64)
    bcp = np.asarray(p["b_cp" if s == 0 else "b_cp_src"], np.float64) + wcp @ bvc
    wm1, bm1 = _fold_ln(p[f"mlp1{sfx}_w"], p[f"ln2{sfx}_w"], p[f"ln2{sfx}_b"])
    bm1 = bm1 + np.asarray(p[f"mlp1{sfx}_b"], np.float64)
    wm2 = np.asarray(p[f"mlp2{sfx}_w"], np.float64)
    bm2 = np.asarray(p[f"mlp2{sfx}_b"], np.float64)

    f16 = lambda a: np.ascontiguousarray(a, np.float16)
    return {
        "tok": np.ascontiguousarray(tok, np.float32),
        "ident": f16(np.eye(128)),
        "ones": f16(np.ones((128, 128))),
        "wqk": f16(wqkv[:2 * C].T),
        "wvs": f16(wqkv[2 * C:].T),
        "wproj": f16(wproj.T),
        "wab": f16(wab.T),
        "wvc": f16(wvc.T),
        "wcp": f16(wcp.T),
        "wm1": f16(wm1.T),
        "wm2": f16(wm2.T),
        "bqk": _pack_bias(bqkv[:2 * C], 12),
        "bab": _pack_bias(bab, 12),
        "bproj": _pack_bias(bproj, 6),
        "bcp": _pack_bias(bcp, 6),
        "bm1": _pack_bias(bm1, 24),
        "bm2": _pack_bias(bm2, 6),
    }


def make_in_maps(inputs):
    x = np.asarray(inputs["x"])
    src = np.asarray(inputs["src"])
    maps = []
    for b in range(B):
        for s in range(2):
            maps.append(_core_inputs(s, x[b] if s == 0 else src[b], inputs))
    return maps


def kernel(**inputs):
    nc = _build()
    in_maps = make_in_maps(inputs)
    res = run_bass_kernel_spmd(nc, in_maps, list(range(N_CORES)))
    x_out = np.stack([res.results[2 * b]["out_tok"] for b in range(B)])
    src_out = np.stack([res.results[2 * b + 1]["out_tok"] for b in range(B)])
    return (x_out.astype(np.float32), src_out.astype(np.float32))


# revision 22
# speedup vs baseline: 1.0759x; 1.0759x over previous
"""CrossAttentionTransformerBlock on 8 TRN2 NeuronCores (Bass/Tile), fp16.

Sharding: core 2b+s handles (batch b, stream s); s=0 -> x, s=1 -> src.
Self-attn + MLP are stream-local; the bidirectional cross-attention exchanges
(keys, values) between the pair (2b, 2b+1) via two chunked pairwise
AllReduces (partner = sum - mine), pipelined behind the tail of the
self-attention block so the wire time is hidden.

All activations/weights are fp16 (PSUM accumulation stays fp32) so every
matmul streams at 1 cycle/row and weight loads use the fast FWL path; the
previous fp32r version ran in fp32_mode=HIGH at less than half throughput
and kept the PE clock throttled. LN scale/bias are folded into the adjacent
projection weights on the host; softmax has no max-subtraction (logits are
small by construction); denominators come from a ones-augmented column of V
in the AV matmul and are applied via a broadcast matmul + wide multiply
(no single-partition reciprocals).
"""

import numpy as np

import concourse.bacc as bacc
import concourse.bass as bass
import concourse.mybir as mybir
import concourse.tile as tile
from concourse.bass_utils import run_bass_kernel_spmd

F32 = mybir.dt.float32
F16 = mybir.dt.float16
AF = mybir.ActivationFunctionType
OP = mybir.AluOpType

B, T, C, H, HD = 4, 1024, 768, 12, 64
HID = 4 * C
EPS = 1e-6
SCALE = HD ** -0.5
NT = T // 128       # 8 token tiles
KC = C // 128       # 6 feature chunks
NH = T // 512       # 2 token halves
NHT = HID // 128    # 24 hidden tiles
N_CORES = 8
GROUPS = [[0, 1], [2, 3], [4, 5], [6, 7]]

# exchange chunk: K-half [C, 512] + V-half [4 x 128, 780], fp16
KCH = C * 512                 # elems of K per chunk
VCH = 4 * 128 * 780           # elems of V per chunk
CCN = KCH + VCH

_CACHE = {}


def _emit(nc):
    dp = nc.declare_dram_parameter
    tok_d = dp("tok", [T, C], F32, isOutput=False)
    ident_d = dp("ident", [128, 128], F16, isOutput=False)
    ones_d = dp("ones", [128, 128], F16, isOutput=False)
    wqk_d = dp("wqk", [C, 2 * C], F16, isOutput=False)
    wvs_d = dp("wvs", [C, C], F16, isOutput=False)
    wproj_d = dp("wproj", [C, C], F16, isOutput=False)
    wab_d = dp("wab", [C, 2 * C], F16, isOutput=False)
    wvc_d = dp("wvc", [C, C], F16, isOutput=False)
    wcp_d = dp("wcp", [C, C], F16, isOutput=False)
    wm1_d = dp("wm1", [C, HID], F16, isOutput=False)
    wm2_d = dp("wm2", [HID, C], F16, isOutput=False)
    bqk_d = dp("bqk", [128, 12], F32, isOutput=False)
    bab_d = dp("bab", [128, 12], F32, isOutput=False)
    bproj_d = dp("bproj", [128, 6], F32, isOutput=False)
    bcp_d = dp("bcp", [128, 6], F32, isOutput=False)
    bm1_d = dp("bm1", [128, 24], F32, isOutput=False)
    bm2_d = dp("bm2", [128, 6], F32, isOutput=False)
    out_d = dp("out_tok", [T, C], F32, isOutput=True)

    with tile.TileContext(nc) as tc:
        import contextlib
        es = contextlib.ExitStack()
        with es:
            es.enter_context(nc.allow_low_precision(
                reason="fp16 kernel by design; matmul/LN accumulate in f32"))
            pP = es.enter_context(tc.tile_pool(name="pP", bufs=1))
            pROW = es.enter_context(tc.tile_pool(name="pROW", bufs=4))
            pSQ = es.enter_context(tc.tile_pool(name="pSQ", bufs=2))
            pVAR = es.enter_context(tc.tile_pool(name="pVAR", bufs=1))
            pPT = es.enter_context(tc.tile_pool(name="pPT", bufs=3))
            pRC = es.enter_context(tc.tile_pool(name="pRC", bufs=1))
            psA = es.enter_context(
                tc.tile_pool(name="psA", bufs=3, space="PSUM"))
            dram = es.enter_context(
                tc.tile_pool(name="dram", bufs=1, space="DRAM"))

            ident = pP.tile([128, 128], F16, tag="ident")
            onesm = pP.tile([128, 128], F16, tag="onesm")
            nc.sync.dma_start(out=ident[:], in_=ident_d[:])
            nc.sync.dma_start(out=onesm[:], in_=ones_d[:])
            eps_t = pP.tile([128, 1], F32, tag="epst")
            nc.vector.memset(eps_t[:], EPS)
            den_tall = [pP.tile([96, T], F16, tag=f"dent{i}",
                                name=f"dent{i}") for i in range(4)]
            for i in range(4):
                nc.vector.memset(den_tall[i][:], 1.0)

            # bias packs
            def bias_pack(b_dram, n):
                bt = pP.tile([128, n], F32, tag=f"B{b_dram.name}", name="bp")
                nc.sync.dma_start(out=bt[:], in_=b_dram[:, 0:n])
                return bt

            bqk_t = bias_pack(bqk_d, 12)
            bab_t = bias_pack(bab_d, 12)
            bproj_t = bias_pack(bproj_d, 6)
            bcp_t = bias_pack(bcp_d, 6)
            bm1_t = bias_pack(bm1_d, 24)
            bm2_t = bias_pack(bm2_d, 6)

            # residual stream + LN output, persistent
            xT = [pP.tile([128, T], F16, tag=f"XT{i}", name=f"xT{i}") for i in range(KC)]
            xh = [pP.tile([128, T], F16, tag=f"XH{i}", name=f"xh{i}") for i in range(KC)]

            # ---- weight slab loaders ----
            def load_slabs(pool, w_dram, width, tag):
                out = []
                for kc in range(KC):
                    s = pool.tile([128, width], F16, tag=f"{tag}{kc}", name=f"w{tag}{kc}")
                    nc.sync.dma_start(
                        out=s[:], in_=w_dram[kc * 128:(kc + 1) * 128, :])
                    out.append(s)
                return out

            # small psum pool: LN stats/broadcasts + 128x128 transposes
            psB = es.enter_context(
                tc.tile_pool(name="psB", bufs=2, space="PSUM"))

            # prefetch-pool stack (LIFO closes; DMAs are emitted later)
            pWpj_cm = tc.tile_pool(name="pWpj", bufs=1)
            pWpj = pWpj_cm.__enter__()
            pWc_cm = tc.tile_pool(name="pWc", bufs=1)
            pWc = pWc_cm.__enter__()
            pOT_cm = tc.tile_pool(name="pOT", bufs=1)
            pOT = pOT_cm.__enter__()
            pWqv_cm = tc.tile_pool(name="pWqv", bufs=1)
            pWqv = pWqv_cm.__enter__()
            pQK_cm = tc.tile_pool(name="pQK", bufs=1)
            pQK = pQK_cm.__enter__()
            pVA_cm = tc.tile_pool(name="pVA", bufs=1)
            pVA = pVA_cm.__enter__()

            # ---- phase 0: tokens -> feature-major fp16 ----
            pTOK_cm = tc.tile_pool(name="pTOK", bufs=2)
            pTOK = pTOK_cm.__enter__()
            pTOKB_cm = tc.tile_pool(name="pTOKB", bufs=2)
            pTOKB = pTOKB_cm.__enter__()
            for tt in range(NT):
                t_ = pTOK.tile([128, C], F32, tag="TOK", name="tok_t")
                nc.sync.dma_start(out=t_[:],
                                  in_=tok_d[tt * 128:(tt + 1) * 128, :])
                tb = pTOKB.tile([128, C], F16, tag="TOKB", name="tokb")
                nc.vector.tensor_copy(tb[:], t_[:])
                for cc in range(KC):
                    pt = psB.tile([128, 128], F16, tag="LN", name="pt")
                    nc.tensor.transpose(
                        pt[:], tb[:, cc * 128:(cc + 1) * 128], ident[:])
                    nc.vector.tensor_copy(
                        xT[cc][:, tt * 128:(tt + 1) * 128], pt[:])
            # weight prefetch: self first, then cross (queue order = need order)
            wqks = load_slabs(pWqv, wqk_d, 2 * C, "WQK")
            wvss = load_slabs(pWqv, wvs_d, C, "WVS")
            wprojs = load_slabs(pWpj, wproj_d, C, "WPJ")
            wabs = load_slabs(pWc, wab_d, 2 * C, "WAB")
            wvcs = load_slabs(pWc, wvc_d, C, "WVC")
            wcps = load_slabs(pWc, wcp_d, C, "WCP")
            pTOKB_cm.__exit__(None, None, None)
            pTOK_cm.__exit__(None, None, None)

            # ---- LN of one token half: writes xh[:][:, hsl] ----
            def ln_half(hf):
                hsl = slice(hf * 512, (hf + 1) * 512)
                ps_mu = psB.tile([1, 512], F32, tag="LN", name="psmu")
                ps_sq = psB.tile([1, 512], F32, tag="LN", name="pssq")
                for kc in range(KC):
                    sqt = pSQ.tile([128, 512], F16, tag="SQT", name="sqt")
                    nc.vector.tensor_tensor(
                        sqt[:], xT[kc][:, hsl], xT[kc][:, hsl], OP.mult)
                    nc.tensor.matmul(
                        ps_mu[0:1, :], onesm[:, 0:1], xT[kc][:, hsl],
                        start=(kc == 0), stop=(kc == KC - 1))
                    nc.tensor.matmul(
                        ps_sq[0:1, :], onesm[:, 0:1], sqt[:],
                        start=(kc == 0), stop=(kc == KC - 1))
                mu_r = pROW.tile([1, 512], F16, tag="ROW", name="mu_r")
                sq_r = pROW.tile([1, 512], F16, tag="ROW", name="sq_r")
                nc.scalar.activation(mu_r[:], ps_mu[0:1, :], AF.Identity,
                                     scale=1.0 / C)
                nc.scalar.activation(sq_r[:], ps_sq[0:1, :], AF.Identity,
                                     scale=1.0 / C)
                mean_b = psB.tile([128, 512], F32, tag="LN", name="meanb")
                m2_b = psB.tile([128, 512], F32, tag="LN", name="m2b")
                nc.tensor.matmul(mean_b[:], onesm[0:1, :], mu_r[0:1, :],
                                 start=True, stop=True)
                nc.tensor.matmul(m2_b[:], onesm[0:1, :], sq_r[0:1, :],
                                 start=True, stop=True)
                meansb = pVAR.tile([128, 512], F32, tag="MEAN", name="meansb")
                nc.vector.tensor_copy(meansb[:], mean_b[:])
                vs = pVAR.tile([128, 512], F32, tag="VAR", name="vs")
                nc.vector.tensor_tensor(vs[:], meansb[:], meansb[:], OP.mult)
                nc.vector.tensor_tensor(vs[:], m2_b[:], vs[:], OP.subtract)
                rstd = pVAR.tile([128, 512], F16, tag="RSTD", name="rstd")
                nc.scalar.activation(vs[:], vs[:], AF.Ln, bias=eps_t[:])
                nc.scalar.activation(rstd[:], vs[:], AF.Exp, scale=-0.5)
                for kc in range(KC):
                    nc.vector.tensor_tensor(
                        xh[kc][:, hsl], xT[kc][:, hsl], meansb[:],
                        OP.subtract)
                    nc.vector.tensor_tensor(
                        xh[kc][:, hsl], xh[kc][:, hsl], rstd[:], OP.mult)

            # ---- feature-major projection of xh into dst tiles ----
            def proj_fm(wslabs, col0, nout, bias_t, bidx0, dst):
                for ot in range(nout):
                    pp = psA.tile([128, T], F32, tag="PS", name="pp")
                    for hf in range(NH):
                        hsl = slice(hf * 512, (hf + 1) * 512)
                        for kc in range(KC):
                            nc.tensor.matmul(
                                pp[:, hsl],
                                wslabs[kc][:, col0 + ot * 128:
                                           col0 + (ot + 1) * 128],
                                xh[kc][:, hsl],
                                start=(kc == 0), stop=(kc == KC - 1))
                    nc.vector.tensor_scalar(
                        dst[ot][:], pp[:], bias_t[:, bidx0 + ot:bidx0 + ot + 1],
                        None, op0=OP.add)

            # ---- token-major ones-augmented V for token tile mt ----
            def proj_v(wslabs, mt, dst):
                pp = psA.tile([128, T], F32, tag="PS", name="ppv")
                for ck in range(2):
                    sl = slice(ck * 512, min((ck + 1) * 512, C))
                    for kc in range(KC):
                        nc.tensor.matmul(
                            pp[:, sl],
                            xh[kc][:, mt * 128:(mt + 1) * 128],
                            wslabs[kc][:, sl],
                            start=(kc == 0), stop=(kc == KC - 1))
                va3 = dst[:].rearrange("p (h e) -> p h e", e=65)
                nc.vector.tensor_copy(
                    va3[:, :, 0:64],
                    pp[:, 0:C].rearrange("p (h e) -> p h e", e=64))
                nc.vector.memset(va3[:, :, 64:65], 1.0)

            # ---- attention: q/k feature-major accessors + V tiles ----
            def norm_head(den_ap, ct, ro, oT):
                dp_ = den_ap.base_partition()
                psR = psA.tile([128, T], F32, tag="PS", name="psR")
                for hf in range(NH):
                    hsl = slice(hf * 512, (hf + 1) * 512)
                    nc.tensor.matmul(psR[0:64, hsl],
                                     onesm[dp_:dp_ + 1, 0:64],
                                     den_ap[:, hsl], start=True, stop=True)
                rcpw = pRC.tile([128, T], F32, tag="RCW", name="rcpw")
                nc.vector.reciprocal_approx_fast(
                    out=rcpw[0:64, :], in_=psR[0:64, :])
                if ro == 64:
                    nc.vector.tensor_copy(rcpw[64:128, :], rcpw[0:64, :])
                nc.vector.tensor_tensor(
                    oT[ct][ro:ro + 64, :], oT[ct][ro:ro + 64, :],
                    rcpw[ro:ro + 64, :], OP.mult)

            def attention(qt_of, kt_of, va_of, pOT):
                oT = [pOT.tile([128, T], F16, tag=f"OT{i}", name=f"oT{i}")
                      for i in range(KC)]
                for h in range(H):
                    ct, ro = divmod(h * HD, 128)
                    psO = psA.tile([128, T], F32, tag="PS", name="psO")
                    for mt in range(NT):
                        psS = psA.tile([128, T], F32, tag="PS", name="psS")
                        for hf in range(NH):
                            hsl = slice(hf * 512, (hf + 1) * 512)
                            nc.tensor.matmul(
                                psS[:, hsl], kt_of(ct, ro, mt),
                                qt_of(ct, ro, hsl),
                                start=True, stop=True)
                        pT = pPT.tile([128, T], F16, tag="PT", name="pT")
                        nc.scalar.activation(pT[:], psS[:], AF.Exp)
                        for hf in range(NH):
                            hsl = slice(hf * 512, (hf + 1) * 512)
                            nc.tensor.matmul(
                                psO[0:65, hsl],
                                va_of(mt)[:, h * 65:(h + 1) * 65],
                                pT[:, hsl],
                                start=(mt == 0), stop=(mt == NT - 1))
                    den = pROW.tile([1, T], F16, tag="DEN", name="den")
                    nc.scalar.activation(den[:], psO[64:65, :], AF.Identity)
                    nc.vector.tensor_copy(oT[ct][ro:ro + 64, :], psO[0:64, :])
                    norm_head(den[0:1, :], ct, ro, oT)
                return oT

            # ---- projection of oT + residual add into xT (half hf) ----
            def proj_res_half(wslabs, bias_t, oT, hf):
                hsl = slice(hf * 512, (hf + 1) * 512)
                for ot in range(KC):
                    pp = psA.tile([128, T], F32, tag="PS", name="ppr")
                    for kc in range(KC):
                        nc.tensor.matmul(
                            pp[:, 0:512],
                            wslabs[kc][:, ot * 128:(ot + 1) * 128],
                            oT[kc][:, hsl],
                            start=(kc == 0), stop=(kc == KC - 1))
                    nc.vector.scalar_tensor_tensor(
                        out=xT[ot][:, hsl], in0=pp[:, 0:512],
                        scalar=bias_t[:, ot:ot + 1], in1=xT[ot][:, hsl],
                        op0=OP.add, op1=OP.add)

            # ================= self-attention =================
            ln_half(0)
            ln_half(1)
            qT = [pQK.tile([128, T], F16, tag=f"QT{i}", name=f"qT{i}") for i in range(KC)]
            kT = [pQK.tile([128, T], F16, tag=f"KT{i}", name=f"kT{i}") for i in range(KC)]
            proj_fm(wqks, 0, KC, bqk_t, 0, qT)
            proj_fm(wqks, C, KC, bqk_t, 6, kT)
            va_s = [pVA.tile([128, 780], F16, tag=f"VA{i}", name=f"va{i}") for i in range(NT)]
            for mt in range(NT):
                proj_v(wvss, mt, va_s[mt])

            oT = attention(
                lambda ct, ro, hsl: qT[ct][ro:ro + HD, hsl],
                lambda ct, ro, mt: kT[ct][ro:ro + HD, mt * 128:(mt + 1) * 128],
                lambda mt: va_s[mt],
                pOT)

            pVA_cm.__exit__(None, None, None)
            pQK_cm.__exit__(None, None, None)
            pWqv_cm.__exit__(None, None, None)

            # ============ cross-attention prep, chunked exchange ============
            pBT_cm = tc.tile_pool(name="pBT", bufs=1)
            pBT = pBT_cm.__enter__()
            pVC_cm = tc.tile_pool(name="pVC", bufs=1)
            pVC = pVC_cm.__enter__()

            bTc = [[pBT.tile([128, 512], F16, tag=f"BT{c}_{i}", name=f"bT{c}_{i}")
                    for i in range(KC)] for c in range(2)]
            bPc = [[pBT.tile([128, 512], F16, tag=f"BP{c}_{i}", name=f"bP{c}_{i}")
                    for i in range(KC)] for c in range(2)]
            aT = [pBT.tile([128, T], F16, tag=f"AT{i}", name=f"aT{i}") for i in range(KC)]
            va_c = [pVC.tile([128, 780], F16, tag=f"VC{i}", name=f"vc{i}") for i in range(NT)]
            vP = [pVC.tile([128, 780], F16, tag=f"VP{i}", name=f"vp{i}") for i in range(NT)]

            cc_in = [dram.tile([CCN], F16, tag=f"cci{c}", name=f"cci{c}") for c in range(2)]
            cc_out = [dram.tile([CCN], F16, tag=f"cco{c}", name=f"cco{c}")
                      for c in range(2)]

            for hf in range(NH):
                hsl = slice(hf * 512, (hf + 1) * 512)
                proj_res_half(wprojs, bproj_t, oT, hf)
                ln_half(hf)
                # bT half: keys this core provides to its partner
                for ot in range(KC):
                    pp = psA.tile([128, T], F32, tag="PS", name="ppb")
                    for kc in range(KC):
                        nc.tensor.matmul(
                            pp[:, 0:512],
                            wabs[kc][:, C + ot * 128:C + (ot + 1) * 128],
                            xh[kc][:, hsl],
                            start=(kc == 0), stop=(kc == KC - 1))
                    nc.vector.tensor_scalar(
                        bTc[hf][ot][:], pp[:, 0:512],
                        bab_t[:, 6 + ot:7 + ot], None, op0=OP.add)
                    nc.sync.dma_start(
                        out=cc_in[hf][ot * 65536:(ot + 1) * 65536]
                        .rearrange("(p c) -> p c", c=512),
                        in_=bTc[hf][ot][:])
                # V half (token tiles of this half)
                for mt in range(hf * 4, hf * 4 + 4):
                    proj_v(wvcs, mt, va_c[mt])
                    nc.sync.dma_start(
                        out=cc_in[hf][KCH + (mt - hf * 4) * 99840:
                                      KCH + (mt - hf * 4 + 1) * 99840]
                        .rearrange("(p c) -> p c", c=780),
                        in_=va_c[mt][:])
                nc.gpsimd.collective_compute(
                    "AllReduce", OP.add, replica_groups=GROUPS,
                    ins=[cc_in[hf].opt()], outs=[cc_out[hf].opt()])

            # queries (overlap with the exchange)
            proj_fm(wabs, 0, KC, bab_t, 0, aT)

            # partner K/V: subtract own contribution from the pair sum
            pRS_cm = tc.tile_pool(name="pRS", bufs=4)
            pRS = pRS_cm.__enter__()
            for c in range(2):
                for ot in range(KC):
                    st = pRS.tile([128, 512], F16, tag="RS", name="st")
                    nc.sync.dma_start(
                        out=st[:],
                        in_=cc_out[c][ot * 65536:(ot + 1) * 65536]
                        .rearrange("(p c) -> p c", c=512))
                    nc.gpsimd.tensor_tensor(
                        bPc[c][ot][:], st[:], bTc[c][ot][:], OP.subtract)
                for mt in range(c * 4, c * 4 + 4):
                    sv = pRS.tile([128, 780], F16, tag="RSV", name="sv")
                    nc.sync.dma_start(
                        out=sv[:],
                        in_=cc_out[c][KCH + (mt - c * 4) * 99840:
                                      KCH + (mt - c * 4 + 1) * 99840]
                        .rearrange("(p c) -> p c", c=780))
                    nc.gpsimd.tensor_tensor(
                        vP[mt][:], sv[:], va_c[mt][:], OP.subtract)

            pOTc_cm = tc.tile_pool(name="pOTc", bufs=1)
            pOTc = pOTc_cm.__enter__()
            oTc = [pOTc.tile([128, T], F16, tag=f"OC{i}", name=f"oC{i}")
                   for i in range(KC)]

            def cross_chunk(c):
                for h in range(H):
                    ct, ro = divmod(h * HD, 128)
                    psO = psA.tile([128, T], F32, tag="PS", name="psOc")
                    for mt in range(c * 4, c * 4 + 4):
                        psS = psA.tile([128, T], F32, tag="PS", name="psSc")
                        for hf in range(NH):
                            hsl = slice(hf * 512, (hf + 1) * 512)
                            nc.tensor.matmul(
                                psS[:, hsl],
                                bPc[c][ct][ro:ro + HD,
                                           (mt % 4) * 128:(mt % 4 + 1) * 128],
                                aT[ct][ro:ro + HD, hsl],
                                start=True, stop=True)
                        pT = pPT.tile([128, T], F16, tag="PT", name="pTc")
                        nc.scalar.activation(pT[:], psS[:], AF.Exp)
                        for hf in range(NH):
                            hsl = slice(hf * 512, (hf + 1) * 512)
                            nc.tensor.matmul(
                                psO[0:65, hsl],
                                vP[mt][:, h * 65:(h + 1) * 65],
                                pT[:, hsl],
                                start=(mt % 4 == 0), stop=(mt % 4 == 3))
                    dt_, dr = den_tall[h // 3], 32 * (h % 3)
                    if c == 0:
                        nc.vector.tensor_copy(oTc[ct][ro:ro + 64, :],
                                              psO[0:64, :])
                        nc.vector.tensor_copy(dt_[dr:dr + 1, :],
                                               psO[64:65, :])
                    else:
                        nc.vector.tensor_tensor(
                            oTc[ct][ro:ro + 64, :], oTc[ct][ro:ro + 64, :],
                            psO[0:64, :], OP.add)
                        nc.vector.tensor_tensor(
                            dt_[dr:dr + 1, :], dt_[dr:dr + 1, :],
                            psO[64:65, :], OP.add)
                        norm_head(dt_[dr:dr + 1, :], ct, ro, oTc)

            cross_chunk(0)
            cross_chunk(1)

            for hf in range(NH):
                proj_res_half(wcps, bcp_t, oTc, hf)

            # free self/cross phase SBUF before MLP (reverse open order)
            pOTc_cm.__exit__(None, None, None)
            pRS_cm.__exit__(None, None, None)
            pVC_cm.__exit__(None, None, None)
            pBT_cm.__exit__(None, None, None)
            pOT_cm.__exit__(None, None, None)
            pWc_cm.__exit__(None, None, None)
            pWpj_cm.__exit__(None, None, None)

            # ================= MLP =================
            pWm1_cm = tc.tile_pool(name="pWm1", bufs=1)
            pWm1 = pWm1_cm.__enter__()
            wm1s = load_slabs(pWm1, wm1_d, HID, "WM1")
            pWm2_cm = tc.tile_pool(name="pWm2", bufs=1)
            pWm2 = pWm2_cm.__enter__()
            wm2s = []
            for ht in range(NHT):
                s = pWm2.tile([128, C], F16, tag=f"WM2_{ht}", name=f"wm2_{ht}")
                nc.sync.dma_start(
                    out=s[:], in_=wm2_d[ht * 128:(ht + 1) * 128, :])
                wm2s.append(s)

            pHT_cm = tc.tile_pool(name="pHT", bufs=1)
            pHT = pHT_cm.__enter__()
            hT = [pHT.tile([128, T], F16, tag=f"HT{i}", name=f"hT{i}") for i in range(NHT)]

            ln_half(0)
            ln_half(1)
            for ht in range(NHT):
                pp = psA.tile([128, T], F32, tag="PS", name="pph")
                for hf in range(NH):
                    hsl = slice(hf * 512, (hf + 1) * 512)
                    for kc in range(KC):
                        nc.tensor.matmul(
                            pp[:, hsl],
                            wm1s[kc][:, ht * 128:(ht + 1) * 128],
                            xh[kc][:, hsl],
                            start=(kc == 0), stop=(kc == KC - 1))
                nc.scalar.activation(hT[ht][:], pp[:], AF.Gelu,
                                     bias=bm1_t[:, ht:ht + 1], scale=1.0)
            for ot in range(KC):
                pp = psA.tile([128, T], F32, tag="PS", name="pp2")
                for hf in range(NH):
                    hsl = slice(hf * 512, (hf + 1) * 512)
                    for ht in range(NHT):
                        nc.tensor.matmul(
                            pp[:, hsl],
                            wm2s[ht][:, ot * 128:(ot + 1) * 128],
                            hT[ht][:, hsl],
                            start=(ht == 0), stop=(ht == NHT - 1))
                nc.vector.scalar_tensor_tensor(
                    out=xT[ot][:], in0=pp[:], scalar=bm2_t[:, ot:ot + 1],
                    in1=xT[ot][:], op0=OP.add, op1=OP.add)

            # ---- output transpose + DMA ----
            pOUT_cm = tc.tile_pool(name="pOUT", bufs=2)
            pOUT = pOUT_cm.__enter__()
            for tt in range(NT):
                ot_sb = pOUT.tile([128, C], F32, tag="OUTT", name="ot_sb")
                for cc in range(KC):
                    pt = psB.tile([128, 128], F16, tag="LN", name="pt2")
                    nc.tensor.transpose(
                        pt[:], xT[cc][:, tt * 128:(tt + 1) * 128], ident[:])
                    nc.vector.tensor_copy(
                        ot_sb[:, cc * 128:(cc + 1) * 128], pt[:])
                nc.sync.dma_start(out=out_d[tt * 128:(tt + 1) * 128, :],
                                  in_=ot_sb[:])
            pOUT_cm.__exit__(None, None, None)
            pHT_cm.__exit__(None, None, None)
            pWm2_cm.__exit__(None, None, None)
            pWm1_cm.__exit__(None, None, None)

    nc.compile()
    return nc


def _build():
    if "nc" not in _CACHE:
        nc = bacc.Bacc("TRN2", target_bir_lowering=False)
        _CACHE["nc"] = _emit(nc)
    return _CACHE["nc"]


def _fold_ln(w, ln_w, ln_b):
    """w [out, in]; returns (w', b') with LN scale/bias folded in."""
    w = np.asarray(w, np.float64)
    wf = w * np.asarray(ln_w, np.float64)[None, :]
    bf = w @ np.asarray(ln_b, np.float64)
    return wf, bf


def _pack_bias(b, n):
    return np.ascontiguousarray(
        np.asarray(b, np.float64).reshape(n, 128).T, np.float32)


def _core_inputs(s, tok, p):
    sfx = "" if s == 0 else "s"
    wqkv, bqkv = _fold_ln(p["w_qkv" + ("" if s == 0 else "_s")],
                          p[f"ln1{sfx}_w"], p[f"ln1{sfx}_b"])
    wqkv = wqkv.copy()
    wqkv[:C] *= SCALE
    bqkv = bqkv.copy()
    bqkv[:C] *= SCALE
    wproj = np.asarray(p["w_proj" + ("" if s == 0 else "_s")], np.float64)
    bproj = np.asarray(p["b_proj" + ("" if s == 0 else "_s")], np.float64) \
        + wproj @ bqkv[2 * C:]
    lncw = p["lnc_w" if s == 0 else "lncs_w"]
    lncb = p["lnc_b" if s == 0 else "lncs_b"]
    wqk, bqk_ = _fold_ln(p["w_qk" if s == 0 else "w_qk_src"], lncw, lncb)
    wqk3 = wqk.reshape(H, 2 * HD, C)
    bqk3 = bqk_.reshape(H, 2 * HD)
    if s == 0:
        A, Ab = wqk3[:, :HD] * SCALE, bqk3[:, :HD] * SCALE
        Bm, Bb = wqk3[:, HD:] * SCALE, bqk3[:, HD:] * SCALE
    else:
        A, Ab = wqk3[:, HD:], bqk3[:, HD:]
        Bm, Bb = wqk3[:, :HD], bqk3[:, :HD]
    wab = np.concatenate([A.reshape(C, C), Bm.reshape(C, C)], axis=0)
    bab = np.concatenate([Ab.reshape(C), Bb.reshape(C)], axis=0)
    wvc, bvc = _fold_ln(p["w_v" if s == 0 else "w_v_src"], lncw, lncb)
    wcp = np.asarray(p["w_cp" if s == 0 else "w_cp_src"], np.float64)
    bcp = np.asarray(p["b_cp" if s == 0 else "b_cp_src"], np.float64) + wcp @ bvc
    wm1, bm1 = _fold_ln(p[f"mlp1{sfx}_w"], p[f"ln2{sfx}_w"], p[f"ln2{sfx}_b"])
    bm1 = bm1 + np.asarray(p[f"mlp1{sfx}_b"], np.float64)
    wm2 = np.asarray(p[f"mlp2{sfx}_w"], np.float64)
    bm2 = np.asarray(p[f"mlp2{sfx}_b"], np.float64)

    f16 = lambda a: np.ascontiguousarray(a, np.float16)
    return {
        "tok": np.ascontiguousarray(tok, np.float32),
        "ident": f16(np.eye(128)),
        "ones": f16(np.ones((128, 128))),
        "wqk": f16(wqkv[:2 * C].T),
        "wvs": f16(wqkv[2 * C:].T),
        "wproj": f16(wproj.T),
        "wab": f16(wab.T),
        "wvc": f16(wvc.T),
        "wcp": f16(wcp.T),
        "wm1": f16(wm1.T),
        "wm2": f16(wm2.T),
        "bqk": _pack_bias(bqkv[:2 * C], 12),
        "bab": _pack_bias(bab, 12),
        "bproj": _pack_bias(bproj, 6),
        "bcp": _pack_bias(bcp, 6),
        "bm1": _pack_bias(bm1, 24),
        "bm2": _pack_bias(bm2, 6),
    }


def make_in_maps(inputs):
    x = np.asarray(inputs["x"])
    src = np.asarray(inputs["src"])
    maps = []
    for b in range(B):
        for s in range(2):
            maps.append(_core_inputs(s, x[b] if s == 0 else src[b], inputs))
    return maps


def kernel(**inputs):
    nc = _build()
    in_maps = make_in_maps(inputs)
    res = run_bass_kernel_spmd(nc, in_maps, list(range(N_CORES)))
    x_out = np.stack([res.results[2 * b]["out_tok"] for b in range(B)])
    src_out = np.stack([res.results[2 * b + 1]["out_tok"] for b in range(B)])
    return (x_out.astype(np.float32), src_out.astype(np.float32))


# revision 23
# speedup vs baseline: 1.1832x; 1.0997x over previous
"""CrossAttentionTransformerBlock on 8 TRN2 NeuronCores (Bass/Tile), fp16.

Sharding: core 2b+s handles (batch b, stream s); s=0 -> x, s=1 -> src.
Self-attn + MLP are stream-local; the bidirectional cross-attention exchanges
(keys, values) between the pair (2b, 2b+1) via two chunked pairwise
AllReduces (partner = sum - mine), pipelined behind the tail of the
self-attention block so the wire time is hidden.

All activations/weights are fp16 (PSUM accumulation stays fp32) so every
matmul streams at 1 cycle/row and weight loads use the fast FWL path; the
previous fp32r version ran in fp32_mode=HIGH at less than half throughput
and kept the PE clock throttled. LN scale/bias are folded into the adjacent
projection weights on the host; softmax has no max-subtraction (logits are
small by construction); denominators come from a ones-augmented column of V
in the AV matmul and are applied via a broadcast matmul + wide multiply
(no single-partition reciprocals).
"""

import numpy as np

import concourse.bacc as bacc
import concourse.bass as bass
import concourse.mybir as mybir
import concourse.tile as tile
from concourse.bass_utils import run_bass_kernel_spmd

F32 = mybir.dt.float32
F16 = mybir.dt.float16
AF = mybir.ActivationFunctionType
OP = mybir.AluOpType

B, T, C, H, HD = 4, 1024, 768, 12, 64
HID = 4 * C
EPS = 1e-6
SCALE = HD ** -0.5
NT = T // 128       # 8 token tiles
KC = C // 128       # 6 feature chunks
NH = T // 512       # 2 token halves
NHT = HID // 128    # 24 hidden tiles
N_CORES = 8
GROUPS = [[0, 1], [2, 3], [4, 5], [6, 7]]

# exchange chunk: K-half [C, 512] + V-half [4 x 128, 780], fp16
KCH = C * 512                 # elems of K per chunk
VCH = 4 * 128 * 780           # elems of V per chunk
CCN = KCH + VCH

_CACHE = {}


def _emit(nc):
    dp = nc.declare_dram_parameter
    tok_d = dp("tok", [T, C], F32, isOutput=False)
    ident_d = dp("ident", [128, 128], F16, isOutput=False)
    ones_d = dp("ones", [128, 128], F16, isOutput=False)
    wqk_d = dp("wqk", [C, 2 * C], F16, isOutput=False)
    wvs_d = dp("wvs", [C, C], F16, isOutput=False)
    wproj_d = dp("wproj", [C, C], F16, isOutput=False)
    wab_d = dp("wab", [C, 2 * C], F16, isOutput=False)
    wvc_d = dp("wvc", [C, C], F16, isOutput=False)
    wcp_d = dp("wcp", [C, C], F16, isOutput=False)
    wm1_d = dp("wm1", [C, HID], F16, isOutput=False)
    wm2_d = dp("wm2", [HID, C], F16, isOutput=False)
    bqk_d = dp("bqk", [128, 12], F32, isOutput=False)
    bab_d = dp("bab", [128, 12], F32, isOutput=False)
    bproj_d = dp("bproj", [128, 6], F32, isOutput=False)
    bcp_d = dp("bcp", [128, 6], F32, isOutput=False)
    bm1_d = dp("bm1", [128, 24], F32, isOutput=False)
    bm2_d = dp("bm2", [128, 6], F32, isOutput=False)
    out_d = dp("out_tok", [T, C], F32, isOutput=True)

    with tile.TileContext(nc) as tc:
        import contextlib
        es = contextlib.ExitStack()
        with es:
            es.enter_context(nc.allow_low_precision(
                reason="fp16 kernel by design; matmul/LN accumulate in f32"))
            pP = es.enter_context(tc.tile_pool(name="pP", bufs=1))
            pROW = es.enter_context(tc.tile_pool(name="pROW", bufs=4))
            pSQ = es.enter_context(tc.tile_pool(name="pSQ", bufs=2))
            pVAR = es.enter_context(tc.tile_pool(name="pVAR", bufs=1))
            pPT = es.enter_context(tc.tile_pool(name="pPT", bufs=3))
            pRC = es.enter_context(tc.tile_pool(name="pRC", bufs=1))
            psA = es.enter_context(
                tc.tile_pool(name="psA", bufs=3, space="PSUM"))
            dram = es.enter_context(
                tc.tile_pool(name="dram", bufs=1, space="DRAM"))

            ident = pP.tile([128, 128], F16, tag="ident")
            onesm = pP.tile([128, 128], F16, tag="onesm")
            nc.sync.dma_start(out=ident[:], in_=ident_d[:])
            nc.sync.dma_start(out=onesm[:], in_=ones_d[:])
            eps_t = pP.tile([128, 1], F32, tag="epst")
            nc.vector.memset(eps_t[:], EPS)
            den_tall = [pP.tile([96, T], F16, tag=f"dent{i}",
                                name=f"dent{i}") for i in range(4)]
            for i in range(4):
                nc.vector.memset(den_tall[i][:], 1.0)

            # bias packs
            def bias_pack(b_dram, n):
                bt = pP.tile([128, n], F32, tag=f"B{b_dram.name}", name="bp")
                nc.sync.dma_start(out=bt[:], in_=b_dram[:, 0:n])
                return bt

            bqk_t = bias_pack(bqk_d, 12)
            bab_t = bias_pack(bab_d, 12)
            bproj_t = bias_pack(bproj_d, 6)
            bcp_t = bias_pack(bcp_d, 6)
            bm1_t = bias_pack(bm1_d, 24)
            bm2_t = bias_pack(bm2_d, 6)

            # residual stream + LN output, persistent
            xT = [pP.tile([128, T], F16, tag=f"XT{i}", name=f"xT{i}") for i in range(KC)]
            xh = [pP.tile([128, T], F16, tag=f"XH{i}", name=f"xh{i}") for i in range(KC)]

            # ---- weight slab loaders ----
            def load_slabs(pool, w_dram, width, tag):
                out = []
                for kc in range(KC):
                    s = pool.tile([128, width], F16, tag=f"{tag}{kc}", name=f"w{tag}{kc}")
                    nc.sync.dma_start(
                        out=s[:], in_=w_dram[kc * 128:(kc + 1) * 128, :])
                    out.append(s)
                return out

            # small psum pool: LN stats/broadcasts + 128x128 transposes
            psB = es.enter_context(
                tc.tile_pool(name="psB", bufs=2, space="PSUM"))

            # prefetch-pool stack (LIFO closes; DMAs are emitted later)
            pWpj_cm = tc.tile_pool(name="pWpj", bufs=1)
            pWpj = pWpj_cm.__enter__()
            pWc_cm = tc.tile_pool(name="pWc", bufs=1)
            pWc = pWc_cm.__enter__()
            pOT_cm = tc.tile_pool(name="pOT", bufs=1)
            pOT = pOT_cm.__enter__()
            pWqv_cm = tc.tile_pool(name="pWqv", bufs=1)
            pWqv = pWqv_cm.__enter__()
            pQK_cm = tc.tile_pool(name="pQK", bufs=1)
            pQK = pQK_cm.__enter__()
            pVA_cm = tc.tile_pool(name="pVA", bufs=1)
            pVA = pVA_cm.__enter__()

            # ---- phase 0: tokens -> feature-major fp16 ----
            pTOK_cm = tc.tile_pool(name="pTOK", bufs=2)
            pTOK = pTOK_cm.__enter__()
            pTOKB_cm = tc.tile_pool(name="pTOKB", bufs=2)
            pTOKB = pTOKB_cm.__enter__()
            for tt in range(NT):
                t_ = pTOK.tile([128, C], F32, tag="TOK", name="tok_t")
                nc.sync.dma_start(out=t_[:],
                                  in_=tok_d[tt * 128:(tt + 1) * 128, :])
                tb = pTOKB.tile([128, C], F16, tag="TOKB", name="tokb")
                nc.vector.tensor_copy(tb[:], t_[:])
                for cc in range(KC):
                    pt = psB.tile([128, 128], F16, tag="LN", name="pt")
                    nc.tensor.transpose(
                        pt[:], tb[:, cc * 128:(cc + 1) * 128], ident[:])
                    nc.vector.tensor_copy(
                        xT[cc][:, tt * 128:(tt + 1) * 128], pt[:])
            # weight prefetch: self first, then cross (queue order = need order)
            wqks = load_slabs(pWqv, wqk_d, 2 * C, "WQK")
            wvss = load_slabs(pWqv, wvs_d, C, "WVS")
            wprojs = load_slabs(pWpj, wproj_d, C, "WPJ")
            wabs = load_slabs(pWc, wab_d, 2 * C, "WAB")
            wvcs = load_slabs(pWc, wvc_d, C, "WVC")
            wcps = load_slabs(pWc, wcp_d, C, "WCP")
            pTOKB_cm.__exit__(None, None, None)
            pTOK_cm.__exit__(None, None, None)

            # ---- LN of one token half: writes xh[:][:, hsl] ----
            def ln_half(hf):
                hsl = slice(hf * 512, (hf + 1) * 512)
                ps_mu = psB.tile([1, 512], F32, tag="LN", name="psmu")
                ps_sq = psB.tile([1, 512], F32, tag="LN", name="pssq")
                for kc in range(KC):
                    sqt = pSQ.tile([128, 512], F16, tag="SQT", name="sqt")
                    nc.vector.tensor_tensor(
                        sqt[:], xT[kc][:, hsl], xT[kc][:, hsl], OP.mult)
                    nc.tensor.matmul(
                        ps_mu[0:1, :], onesm[:, 0:1], xT[kc][:, hsl],
                        start=(kc == 0), stop=(kc == KC - 1))
                    nc.tensor.matmul(
                        ps_sq[0:1, :], onesm[:, 0:1], sqt[:],
                        start=(kc == 0), stop=(kc == KC - 1))
                mu_r = pROW.tile([1, 512], F16, tag="ROW", name="mu_r")
                sq_r = pROW.tile([1, 512], F16, tag="ROW", name="sq_r")
                nc.scalar.activation(mu_r[:], ps_mu[0:1, :], AF.Identity,
                                     scale=1.0 / C)
                nc.scalar.activation(sq_r[:], ps_sq[0:1, :], AF.Identity,
                                     scale=1.0 / C)
                mean_b = psB.tile([128, 512], F32, tag="LN", name="meanb")
                m2_b = psB.tile([128, 512], F32, tag="LN", name="m2b")
                nc.tensor.matmul(mean_b[:], onesm[0:1, :], mu_r[0:1, :],
                                 start=True, stop=True)
                nc.tensor.matmul(m2_b[:], onesm[0:1, :], sq_r[0:1, :],
                                 start=True, stop=True)
                meansb = pVAR.tile([128, 512], F32, tag="MEAN", name="meansb")
                nc.vector.tensor_copy(meansb[:], mean_b[:])
                vs = pVAR.tile([128, 512], F32, tag="VAR", name="vs")
                nc.vector.tensor_tensor(vs[:], meansb[:], meansb[:], OP.mult)
                nc.vector.tensor_tensor(vs[:], m2_b[:], vs[:], OP.subtract)
                rstd = pVAR.tile([128, 512], F16, tag="RSTD", name="rstd")
                nc.scalar.activation(vs[:], vs[:], AF.Ln, bias=eps_t[:])
                nc.scalar.activation(rstd[:], vs[:], AF.Exp, scale=-0.5)
                for kc in range(KC):
                    nc.vector.tensor_tensor(
                        xh[kc][:, hsl], xT[kc][:, hsl], meansb[:],
                        OP.subtract)
                    nc.vector.tensor_tensor(
                        xh[kc][:, hsl], xh[kc][:, hsl], rstd[:], OP.mult)

            # ---- feature-major projection of xh into dst tiles ----
            def proj_fm(wslabs, col0, nout, bias_t, bidx0, dst):
                for ot in range(nout):
                    pp = psA.tile([128, T], F32, tag="PS", name="pp")
                    for hf in range(NH):
                        hsl = slice(hf * 512, (hf + 1) * 512)
                        for kc in range(KC):
                            nc.tensor.matmul(
                                pp[:, hsl],
                                wslabs[kc][:, col0 + ot * 128:
                                           col0 + (ot + 1) * 128],
                                xh[kc][:, hsl],
                                start=(kc == 0), stop=(kc == KC - 1))
                    nc.vector.tensor_scalar(
                        dst[ot][:], pp[:], bias_t[:, bidx0 + ot:bidx0 + ot + 1],
                        None, op0=OP.add)

            # ---- token-major ones-augmented V for token tile mt ----
            def proj_v(wslabs, mt, dst):
                pp = psA.tile([128, T], F32, tag="PS", name="ppv")
                for ck in range(2):
                    sl = slice(ck * 512, min((ck + 1) * 512, C))
                    for kc in range(KC):
                        nc.tensor.matmul(
                            pp[:, sl],
                            xh[kc][:, mt * 128:(mt + 1) * 128],
                            wslabs[kc][:, sl],
                            start=(kc == 0), stop=(kc == KC - 1))
                va3 = dst[:].rearrange("p (h e) -> p h e", e=65)
                nc.vector.tensor_copy(
                    va3[:, :, 0:64],
                    pp[:, 0:C].rearrange("p (h e) -> p h e", e=64))
                nc.vector.memset(va3[:, :, 64:65], 1.0)

            # ---- attention: q/k feature-major accessors + V tiles ----
            def norm_head(h, hsl, cols, psO, oT):
                ct, ro = divmod(h * HD, 128)
                dt_, dr = den_tall[h // 3], 32 * (h % 3)
                nc.vector.tensor_copy(dt_[dr:dr + 1, hsl],
                                      psO[64:65, cols])
                nc.vector.tensor_copy(oT[ct][ro:ro + 64, hsl],
                                      psO[0:64, cols])
                psR = psA.tile([128, T], F32, tag="PS", name="psR")
                nc.tensor.matmul(psR[0:64, 0:512], onesm[dr:dr + 1, 0:64],
                                 dt_[dr:dr + 1, hsl], start=True, stop=True)
                rcpw = pRC.tile([128, 512], F32, tag="RCW", name="rcpw")
                nc.vector.reciprocal_approx_fast(
                    out=rcpw[0:64, :], in_=psR[0:64, 0:512])
                if ro == 64:
                    nc.vector.tensor_copy(rcpw[64:128, :], rcpw[0:64, :])
                nc.vector.tensor_tensor(
                    oT[ct][ro:ro + 64, hsl], oT[ct][ro:ro + 64, hsl],
                    rcpw[ro:ro + 64, :], OP.mult)

            def attention(qt_of, kt_of, va_of, pOT):
                oT = [pOT.tile([128, T], F16, tag=f"OT{i}", name=f"oT{i}")
                      for i in range(KC)]
                for hf in range(NH):
                    hsl = slice(hf * 512, (hf + 1) * 512)
                    for ct in range(KC):
                        hA, hB = 2 * ct, 2 * ct + 1
                        psO = psA.tile([128, T], F32, tag="PS", name="psO")
                        for mt in range(NT):
                            psS = psA.tile([128, T], F32, tag="PS",
                                           name="psS")
                            nc.tensor.matmul(
                                psS[:, 0:512], kt_of(ct, 0, mt),
                                qt_of(ct, 0, hsl),
                                start=True, stop=True,
                                tile_position=(0, 0))
                            nc.tensor.matmul(
                                psS[:, 512:1024], kt_of(ct, 64, mt),
                                qt_of(ct, 64, hsl),
                                start=True, stop=True,
                                tile_position=(64, 0))
                            pT = pPT.tile([128, T], F16, tag="PT", name="pT")
                            nc.scalar.activation(pT[:], psS[:], AF.Exp)
                            nc.tensor.matmul(
                                psO[0:65, 0:512],
                                va_of(mt)[:, hA * 65:(hA + 1) * 65],
                                pT[:, 0:512],
                                start=(mt == 0), stop=(mt == NT - 1))
                            nc.tensor.matmul(
                                psO[0:65, 512:1024],
                                va_of(mt)[:, hB * 65:(hB + 1) * 65],
                                pT[:, 512:1024],
                                start=(mt == 0), stop=(mt == NT - 1))
                        norm_head(hA, hsl, slice(0, 512), psO, oT)
                        norm_head(hB, hsl, slice(512, 1024), psO, oT)
                return oT

            # ---- projection of oT + residual add into xT (half hf) ----
            def proj_res_half(wslabs, bias_t, oT, hf):
                hsl = slice(hf * 512, (hf + 1) * 512)
                for ot in range(KC):
                    pp = psA.tile([128, T], F32, tag="PS", name="ppr")
                    for kc in range(KC):
                        nc.tensor.matmul(
                            pp[:, 0:512],
                            wslabs[kc][:, ot * 128:(ot + 1) * 128],
                            oT[kc][:, hsl],
                            start=(kc == 0), stop=(kc == KC - 1))
                    nc.vector.scalar_tensor_tensor(
                        out=xT[ot][:, hsl], in0=pp[:, 0:512],
                        scalar=bias_t[:, ot:ot + 1], in1=xT[ot][:, hsl],
                        op0=OP.add, op1=OP.add)

            # ================= self-attention =================
            ln_half(0)
            ln_half(1)
            qT = [pQK.tile([128, T], F16, tag=f"QT{i}", name=f"qT{i}") for i in range(KC)]
            kT = [pQK.tile([128, T], F16, tag=f"KT{i}", name=f"kT{i}") for i in range(KC)]
            proj_fm(wqks, 0, KC, bqk_t, 0, qT)
            proj_fm(wqks, C, KC, bqk_t, 6, kT)
            va_s = [pVA.tile([128, 780], F16, tag=f"VA{i}", name=f"va{i}") for i in range(NT)]
            for mt in range(NT):
                proj_v(wvss, mt, va_s[mt])

            oT = attention(
                lambda ct, ro, hsl: qT[ct][ro:ro + HD, hsl],
                lambda ct, ro, mt: kT[ct][ro:ro + HD, mt * 128:(mt + 1) * 128],
                lambda mt: va_s[mt],
                pOT)

            pVA_cm.__exit__(None, None, None)
            pQK_cm.__exit__(None, None, None)
            pWqv_cm.__exit__(None, None, None)

            # ============ cross-attention prep, chunked exchange ============
            pBT_cm = tc.tile_pool(name="pBT", bufs=1)
            pBT = pBT_cm.__enter__()
            pVC_cm = tc.tile_pool(name="pVC", bufs=1)
            pVC = pVC_cm.__enter__()

            bTc = [[pBT.tile([128, 512], F16, tag=f"BT{c}_{i}", name=f"bT{c}_{i}")
                    for i in range(KC)] for c in range(2)]
            bPc = [[pBT.tile([128, 512], F16, tag=f"BP{c}_{i}", name=f"bP{c}_{i}")
                    for i in range(KC)] for c in range(2)]
            aT = [pBT.tile([128, T], F16, tag=f"AT{i}", name=f"aT{i}") for i in range(KC)]
            va_c = [pVC.tile([128, 780], F16, tag=f"VC{i}", name=f"vc{i}") for i in range(NT)]
            vP = [pVC.tile([128, 780], F16, tag=f"VP{i}", name=f"vp{i}") for i in range(NT)]

            cc_in = [dram.tile([CCN], F16, tag=f"cci{c}", name=f"cci{c}") for c in range(2)]
            cc_out = [dram.tile([CCN], F16, tag=f"cco{c}", name=f"cco{c}")
                      for c in range(2)]

            for hf in range(NH):
                hsl = slice(hf * 512, (hf + 1) * 512)
                proj_res_half(wprojs, bproj_t, oT, hf)
                ln_half(hf)
                # bT half: keys this core provides to its partner
                for ot in range(KC):
                    pp = psA.tile([128, T], F32, tag="PS", name="ppb")
                    for kc in range(KC):
                        nc.tensor.matmul(
                            pp[:, 0:512],
                            wabs[kc][:, C + ot * 128:C + (ot + 1) * 128],
                            xh[kc][:, hsl],
                            start=(kc == 0), stop=(kc == KC - 1))
                    nc.vector.tensor_scalar(
                        bTc[hf][ot][:], pp[:, 0:512],
                        bab_t[:, 6 + ot:7 + ot], None, op0=OP.add)
                    nc.sync.dma_start(
                        out=cc_in[hf][ot * 65536:(ot + 1) * 65536]
                        .rearrange("(p c) -> p c", c=512),
                        in_=bTc[hf][ot][:])
                # V half (token tiles of this half)
                for mt in range(hf * 4, hf * 4 + 4):
                    proj_v(wvcs, mt, va_c[mt])
                    nc.sync.dma_start(
                        out=cc_in[hf][KCH + (mt - hf * 4) * 99840:
                                      KCH + (mt - hf * 4 + 1) * 99840]
                        .rearrange("(p c) -> p c", c=780),
                        in_=va_c[mt][:])
                nc.gpsimd.collective_compute(
                    "AllReduce", OP.add, replica_groups=GROUPS,
                    ins=[cc_in[hf].opt()], outs=[cc_out[hf].opt()])

            # queries (overlap with the exchange)
            proj_fm(wabs, 0, KC, bab_t, 0, aT)

            # partner K/V: subtract own contribution from the pair sum
            pRS_cm = tc.tile_pool(name="pRS", bufs=4)
            pRS = pRS_cm.__enter__()
            for c in range(2):
                for ot in range(KC):
                    st = pRS.tile([128, 512], F16, tag="RS", name="st")
                    nc.sync.dma_start(
                        out=st[:],
                        in_=cc_out[c][ot * 65536:(ot + 1) * 65536]
                        .rearrange("(p c) -> p c", c=512))
                    nc.gpsimd.tensor_tensor(
                        bPc[c][ot][:], st[:], bTc[c][ot][:], OP.subtract)
                for mt in range(c * 4, c * 4 + 4):
                    sv = pRS.tile([128, 780], F16, tag="RSV", name="sv")
                    nc.sync.dma_start(
                        out=sv[:],
                        in_=cc_out[c][KCH + (mt - c * 4) * 99840:
                                      KCH + (mt - c * 4 + 1) * 99840]
                        .rearrange("(p c) -> p c", c=780))
                    nc.gpsimd.tensor_tensor(
                        vP[mt][:], sv[:], va_c[mt][:], OP.subtract)

            pOTc_cm = tc.tile_pool(name="pOTc", bufs=1)
            pOTc = pOTc_cm.__enter__()
            oTc = attention(
                lambda ct, ro, hsl: aT[ct][ro:ro + HD, hsl],
                lambda ct, ro, mt: bPc[mt // 4][ct][ro:ro + HD,
                                                    (mt % 4) * 128:
                                                    (mt % 4 + 1) * 128],
                lambda mt: vP[mt],
                pOTc)

            for hf in range(NH):
                proj_res_half(wcps, bcp_t, oTc, hf)

            # free self/cross phase SBUF before MLP (reverse open order)
            pOTc_cm.__exit__(None, None, None)
            pRS_cm.__exit__(None, None, None)
            pVC_cm.__exit__(None, None, None)
            pBT_cm.__exit__(None, None, None)
            pOT_cm.__exit__(None, None, None)
            pWc_cm.__exit__(None, None, None)
            pWpj_cm.__exit__(None, None, None)

            # ================= MLP =================
            pWm1_cm = tc.tile_pool(name="pWm1", bufs=1)
            pWm1 = pWm1_cm.__enter__()
            wm1s = load_slabs(pWm1, wm1_d, HID, "WM1")
            pWm2_cm = tc.tile_pool(name="pWm2", bufs=1)
            pWm2 = pWm2_cm.__enter__()
            wm2s = []
            for ht in range(NHT):
                s = pWm2.tile([128, C], F16, tag=f"WM2_{ht}", name=f"wm2_{ht}")
                nc.sync.dma_start(
                    out=s[:], in_=wm2_d[ht * 128:(ht + 1) * 128, :])
                wm2s.append(s)

            pHT_cm = tc.tile_pool(name="pHT", bufs=1)
            pHT = pHT_cm.__enter__()
            hT = [pHT.tile([128, T], F16, tag=f"HT{i}", name=f"hT{i}") for i in range(NHT)]

            ln_half(0)
            ln_half(1)
            for ht in range(NHT):
                pp = psA.tile([128, T], F32, tag="PS", name="pph")
                for hf in range(NH):
                    hsl = slice(hf * 512, (hf + 1) * 512)
                    for kc in range(KC):
                        nc.tensor.matmul(
                            pp[:, hsl],
                            wm1s[kc][:, ht * 128:(ht + 1) * 128],
                            xh[kc][:, hsl],
                            start=(kc == 0), stop=(kc == KC - 1))
                nc.scalar.activation(hT[ht][:], pp[:], AF.Gelu,
                                     bias=bm1_t[:, ht:ht + 1], scale=1.0)
            for ot in range(KC):
                pp = psA.tile([128, T], F32, tag="PS", name="pp2")
                for hf in range(NH):
                    hsl = slice(hf * 512, (hf + 1) * 512)
                    for ht in range(NHT):
                        nc.tensor.matmul(
                            pp[:, hsl],
                            wm2s[ht][:, ot * 128:(ot + 1) * 128],
                            hT[ht][:, hsl],
                            start=(ht == 0), stop=(ht == NHT - 1))
                nc.vector.scalar_tensor_tensor(
                    out=xT[ot][:], in0=pp[:], scalar=bm2_t[:, ot:ot + 1],
                    in1=xT[ot][:], op0=OP.add, op1=OP.add)

            # ---- output transpose + DMA ----
            pOUT_cm = tc.tile_pool(name="pOUT", bufs=2)
            pOUT = pOUT_cm.__enter__()
            for tt in range(NT):
                ot_sb = pOUT.tile([128, C], F32, tag="OUTT", name="ot_sb")
                for cc in range(KC):
                    pt = psB.tile([128, 128], F16, tag="LN", name="pt2")
                    nc.tensor.transpose(
                        pt[:], xT[cc][:, tt * 128:(tt + 1) * 128], ident[:])
                    nc.vector.tensor_copy(
                        ot_sb[:, cc * 128:(cc + 1) * 128], pt[:])
                nc.sync.dma_start(out=out_d[tt * 128:(tt + 1) * 128, :],
                                  in_=ot_sb[:])
            pOUT_cm.__exit__(None, None, None)
            pHT_cm.__exit__(None, None, None)
            pWm2_cm.__exit__(None, None, None)
            pWm1_cm.__exit__(None, None, None)

    nc.compile()
    return nc


def _build():
    if "nc" not in _CACHE:
        nc = bacc.Bacc("TRN2", target_bir_lowering=False)
        _CACHE["nc"] = _emit(nc)
    return _CACHE["nc"]


def _fold_ln(w, ln_w, ln_b):
    """w [out, in]; returns (w', b') with LN scale/bias folded in."""
    w = np.asarray(w, np.float64)
    wf = w * np.asarray(ln_w, np.float64)[None, :]
    bf = w @ np.asarray(ln_b, np.float64)
    return wf, bf


def _pack_bias(b, n):
    return np.ascontiguousarray(
        np.asarray(b, np.float64).reshape(n, 128).T, np.float32)


def _core_inputs(s, tok, p):
    sfx = "" if s == 0 else "s"
    wqkv, bqkv = _fold_ln(p["w_qkv" + ("" if s == 0 else "_s")],
                          p[f"ln1{sfx}_w"], p[f"ln1{sfx}_b"])
    wqkv = wqkv.copy()
    wqkv[:C] *= SCALE
    bqkv = bqkv.copy()
    bqkv[:C] *= SCALE
    wproj = np.asarray(p["w_proj" + ("" if s == 0 else "_s")], np.float64)
    bproj = np.asarray(p["b_proj" + ("" if s == 0 else "_s")], np.float64) \
        + wproj @ bqkv[2 * C:]
    lncw = p["lnc_w" if s == 0 else "lncs_w"]
    lncb = p["lnc_b" if s == 0 else "lncs_b"]
    wqk, bqk_ = _fold_ln(p["w_qk" if s == 0 else "w_qk_src"], lncw, lncb)
    wqk3 = wqk.reshape(H, 2 * HD, C)
    bqk3 = bqk_.reshape(H, 2 * HD)
    if s == 0:
        A, Ab = wqk3[:, :HD] * SCALE, bqk3[:, :HD] * SCALE
        Bm, Bb = wqk3[:, HD:] * SCALE, bqk3[:, HD:] * SCALE
    else:
        A, Ab = wqk3[:, HD:], bqk3[:, HD:]
        Bm, Bb = wqk3[:, :HD], bqk3[:, :HD]
    wab = np.concatenate([A.reshape(C, C), Bm.reshape(C, C)], axis=0)
    bab = np.concatenate([Ab.reshape(C), Bb.reshape(C)], axis=0)
    wvc, bvc = _fold_ln(p["w_v" if s == 0 else "w_v_src"], lncw, lncb)
    wcp = np.asarray(p["w_cp" if s == 0 else "w_cp_src"], np.float64)
    bcp = np.asarray(p["b_cp" if s == 0 else "b_cp_src"], np.float64) + wcp @ bvc
    wm1, bm1 = _fold_ln(p[f"mlp1{sfx}_w"], p[f"ln2{sfx}_w"], p[f"ln2{sfx}_b"])
    bm1 = bm1 + np.asarray(p[f"mlp1{sfx}_b"], np.float64)
    wm2 = np.asarray(p[f"mlp2{sfx}_w"], np.float64)
    bm2 = np.asarray(p[f"mlp2{sfx}_b"], np.float64)

    f16 = lambda a: np.ascontiguousarray(a, np.float16)
    return {
        "tok": np.ascontiguousarray(tok, np.float32),
        "ident": f16(np.eye(128)),
        "ones": f16(np.ones((128, 128))),
        "wqk": f16(wqkv[:2 * C].T),
        "wvs": f16(wqkv[2 * C:].T),
        "wproj": f16(wproj.T),
        "wab": f16(wab.T),
        "wvc": f16(wvc.T),
        "wcp": f16(wcp.T),
        "wm1": f16(wm1.T),
        "wm2": f16(wm2.T),
        "bqk": _pack_bias(bqkv[:2 * C], 12),
        "bab": _pack_bias(bab, 12),
        "bproj": _pack_bias(bproj, 6),
        "bcp": _pack_bias(bcp, 6),
        "bm1": _pack_bias(bm1, 24),
        "bm2": _pack_bias(bm2, 6),
    }


def make_in_maps(inputs):
    x = np.asarray(inputs["x"])
    src = np.asarray(inputs["src"])
    maps = []
    for b in range(B):
        for s in range(2):
            maps.append(_core_inputs(s, x[b] if s == 0 else src[b], inputs))
    return maps


def kernel(**inputs):
    nc = _build()
    in_maps = make_in_maps(inputs)
    res = run_bass_kernel_spmd(nc, in_maps, list(range(N_CORES)))
    x_out = np.stack([res.results[2 * b]["out_tok"] for b in range(B)])
    src_out = np.stack([res.results[2 * b + 1]["out_tok"] for b in range(B)])
    return (x_out.astype(np.float32), src_out.astype(np.float32))
